# revision 2
# baseline (speedup 1.0000x reference)
"""NonLocalBlock (B=8, C=256, HW=64x64) Trainium2 kernel.

Strategy: data-parallel over batch, one sample per NeuronCore (8 cores).
Per core (all on-chip, the [N,N]=67MB attention matrix never touches HBM):

  x [C=256, N=4096] --DMA--> SBUF, cast to fp32r (logit path) + bf16 (g path)
  theta = w_theta @ x + b_theta    [O=128, N]  fp32r   (PE, fp32r)
  phi   = w_phi   @ x + b_phi      [O=128, N]  fp32r   (PE, fp32r)
  gt    = (w_g @ x)^T              [N, O] bf16 chunks + ones column (PE, bf16)
  for each n-tile (512 cols):
    S^T chunks [m=128, n=512] = phi_chunk^T . theta_tile   (PE fp32r, PSUM)
    P^T = exp(S^T)  (ScalarE, PSUM->SBUF bf16; no max-subtract: logits <~50)
    y [n-sub=128, 129] += P^T_chunk^T . [gt_chunk | ones]  (PE bf16, PSUM)
      -> col 128 is the softmax row-sum (free)
    ynorm = y[:, :128] * (1/y[:,128])  (DVE, bf16)
    yt [o, n] = transpose(ynorm) + b_g  (PE transpose + DVE bias, bf16)
    z [c-tile, 512] = w_out_tile^T . yt  (PE bf16)
    out = x + z*bn_scale + bn_shift  (DVE), DMA out

Softmax needs no max subtraction: logits std ~7.2, |max| <~ 55 over 16.7M
samples, exp(55)=7.7e23 and row sums < 4096*exp(55) stay far below fp32/bf16
max (3.4e38). b_phi is kept (exact); the bias of g is applied after
normalization (softmax rows sum to 1, so it commutes).
"""
import os
import sys

sys.path.insert(0, "/opt/trn_rl_repo")

import numpy as np
import ml_dtypes

import concourse.bass as bass
import concourse.bacc as bacc
import concourse.mybir as mybir
import concourse.tile as tile
from concourse.bass_utils import run_bass_kernel_spmd
from concourse.masks import make_identity

F32 = mybir.dt.float32
F32R = mybir.dt.float32r
BF16 = mybir.dt.bfloat16
ADD = mybir.AluOpType.add
MULT = mybir.AluOpType.mult

B, C, O, N = 8, 256, 128, 4096
NT = 512          # n-tile width (PSUM bank / matmul free-dim limit)
N_TILES = N // NT           # 8
M_CHUNKS = N // 128         # 32
GRPS = M_CHUNKS // 4        # 8 groups of 4 m-chunks share one exp ACTIVATE
BN_EPS = 1e-5


def build_nc():
    nc = bacc.Bacc()

    xin = nc.dram_tensor("xin", [C, N], F32, kind="ExternalInput")
    wth = nc.dram_tensor("wth", [C, O], F32, kind="ExternalInput")     # w_theta.T
    wph = nc.dram_tensor("wph", [C, O], F32, kind="ExternalInput")     # w_phi.T
    wg = nc.dram_tensor("wg", [C, O], BF16, kind="ExternalInput")      # w_g.T
    wout = nc.dram_tensor("wout", [O, C], BF16, kind="ExternalInput")  # w_out.T
    bth = nc.dram_tensor("bth", [O, 1], F32, kind="ExternalInput")
    bph = nc.dram_tensor("bph", [O, 1], F32, kind="ExternalInput")
    bg = nc.dram_tensor("bg", [O, 1], F32, kind="ExternalInput")
    bnscale = nc.dram_tensor("bnscale", [128, 2], F32, kind="ExternalInput")
    bnshift = nc.dram_tensor("bnshift", [128, 2], F32, kind="ExternalInput")
    out = nc.dram_tensor("out", [C, N], F32, kind="ExternalOutput")

    with tile.TileContext(nc) as tc:
        with tc.tile_pool(name="const", bufs=1) as const, \
             tc.tile_pool(name="xpool", bufs=1) as xpool, \
             tc.tile_pool(name="proj", bufs=1) as proj, \
             tc.tile_pool(name="yt_pool", bufs=3) as yt_pool, \
             tc.tile_pool(name="small", bufs=4) as small, \
             tc.tile_pool(name="ostage", bufs=4) as ostage:

            # ---- constants ----
            wth_st = const.tile([128, 2 * O], F32)
            wph_st = const.tile([128, 2 * O], F32)
            for k in range(2):
                nc.sync.dma_start(wth_st[:, k * O:(k + 1) * O], wth[k * 128:(k + 1) * 128, :])
                nc.sync.dma_start(wph_st[:, k * O:(k + 1) * O], wph[k * 128:(k + 1) * 128, :])
            wth_r = const.tile([128, 2 * O], F32R)
            wph_r = const.tile([128, 2 * O], F32R)
            nc.vector.tensor_copy(wth_r[:], wth_st[:])
            nc.vector.tensor_copy(wph_r[:], wph_st[:])
            wg_sb = const.tile([128, 2 * O], BF16)
            for k in range(2):
                nc.sync.dma_start(wg_sb[:, k * O:(k + 1) * O], wg[k * 128:(k + 1) * 128, :])
            wout_sb = const.tile([O, C], BF16)
            nc.sync.dma_start(wout_sb[:], wout[:])
            bth_sb = const.tile([O, 1], F32)
            bph_sb = const.tile([O, 1], F32)
            bg_sb = const.tile([O, 1], F32)
            bnscale_sb = const.tile([128, 2], F32)
            bnshift_sb = const.tile([128, 2], F32)
            nc.sync.dma_start(bth_sb[:], bth[:])
            nc.sync.dma_start(bph_sb[:], bph[:])
            nc.sync.dma_start(bg_sb[:], bg[:])
            nc.sync.dma_start(bnscale_sb[:], bnscale[:])
            nc.sync.dma_start(bnshift_sb[:], bnshift[:])
            ident = const.tile([128, 128], BF16)
            make_identity(nc, ident)

            # ---- x: load + casts (fp32r for logit path, bf16 for g path) ----
            # x_r also serves as the residual source (fp32r == rounded fp32).
            x_r = [xpool.tile([128, N], F32R, name=f"x{i}_r") for i in range(2)]
            x_bf = [xpool.tile([128, N], BF16, name=f"x{i}_bf") for i in range(2)]

            theta_r = proj.tile([O, N], F32R)
            phi_r = proj.tile([O, N], F32R)
            gt_sb = proj.tile([128, M_CHUNKS * (O + 1)], BF16)  # [m,o]+ones col

            with tc.tile_pool(name="xstage", bufs=4) as xstage, \
                 tc.tile_pool(name="thph_ps", bufs=1, space="PSUM") as thph_ps, \
                 tc.tile_pool(name="gt_ps", bufs=2, space="PSUM") as gt_ps:

                for i in range(2):
                    for h in range(2):
                        xs = xstage.tile([128, N // 2], F32)
                        hsl = slice(h * (N // 2), (h + 1) * (N // 2))
                        nc.sync.dma_start(xs[:], xin[i * 128:(i + 1) * 128, hsl])
                        nc.vector.tensor_copy(x_r[i][:, hsl], xs[:])
                        nc.vector.tensor_copy(x_bf[i][:, hsl], xs[:])

                # phi first (S^T needs all of phi), then theta, then gt
                for (dst, w_r, b_sb) in ((phi_r, wph_r, bph_sb), (theta_r, wth_r, bth_sb)):
                    for h in range(2):  # halves of N, 2048 each
                        pp = thph_ps.tile([128, N // 2], F32)
                        for j in range(4):  # 512-wide matmuls
                            nsl_ps = slice(j * NT, (j + 1) * NT)
                            nsl = slice(h * (N // 2) + j * NT, h * (N // 2) + (j + 1) * NT)
                            for k in range(2):
                                nc.tensor.matmul(
                                    pp[:, nsl_ps],
                                    w_r[:, k * O:(k + 1) * O],
                                    x_r[k][:, nsl],
                                    start=(k == 0), stop=(k == 1),
                                )
                        hsl = slice(h * (N // 2), (h + 1) * (N // 2))
                        nc.vector.tensor_scalar_add(dst[:, hsl], pp[:], b_sb[:])

                for c in range(M_CHUNKS):
                    gp = gt_ps.tile([128, O], F32)
                    for k in range(2):
                        nc.tensor.matmul(
                            gp[:],
                            x_bf[k][:, c * 128:(c + 1) * 128],
                            wg_sb[:, k * O:(k + 1) * O],
                            start=(k == 0), stop=(k == 1),
                        )
                    base = c * (O + 1)
                    nc.vector.tensor_copy(gt_sb[:, base:base + O], gp[:])
                    nc.gpsimd.memset(gt_sb[:, base + O:base + O + 1], 1.0)

            # ---- attention + output ----
            with tc.tile_pool(name="pt_pool", bufs=10) as pt_pool, \
                 tc.tile_pool(name="st_ps", bufs=1, space="PSUM") as st_ps, \
                 tc.tile_pool(name="y_ps", bufs=2, space="PSUM") as y_ps, \
                 tc.tile_pool(name="yt_ps", bufs=1, space="PSUM") as yt_ps, \
                 tc.tile_pool(name="o_ps", bufs=1, space="PSUM") as o_ps:

                for nt in range(N_TILES):
                    ntsl = slice(nt * NT, (nt + 1) * NT)
                    pts = []
                    for g in range(GRPS):
                        st = st_ps.tile([128, 4 * NT], F32)
                        for k in range(4):
                            c = 4 * g + k
                            nc.tensor.matmul(
                                st[:, k * NT:(k + 1) * NT],
                                phi_r[:, c * 128:(c + 1) * 128],
                                theta_r[:, ntsl],
                                start=True, stop=True,
                            )
                        pt = pt_pool.tile([128, 4 * NT], BF16)
                        nc.scalar.activation(pt[:], st[:], mybir.ActivationFunctionType.Exp)
                        pts.append(pt)

                    yt_sb = yt_pool.tile([O, NT], BF16)
                    for s in range(4):
                        y = y_ps.tile([128, O + 1], F32)
                        for g in range(GRPS):
                            for k in range(4):
                                c = 4 * g + k
                                nc.tensor.matmul(
                                    y[:],
                                    pts[g][:, k * NT + s * 128:k * NT + (s + 1) * 128],
                                    gt_sb[:, c * (O + 1):(c + 1) * (O + 1)],
                                    start=(c == 0), stop=(c == M_CHUNKS - 1),
                                )
                        recip = small.tile([128, 1], F32)
                        nc.vector.reciprocal(recip[:], y[:, O:O + 1])
                        ynorm = small.tile([128, O], BF16)
                        nc.vector.tensor_scalar_mul(ynorm[:], y[:, 0:O], recip[:])
                        ytp = yt_ps.tile([128, 128], BF16)
                        nc.tensor.transpose(ytp[:], ynorm[:], ident[:])
                        nc.vector.tensor_scalar_add(yt_sb[:, s * 128:(s + 1) * 128], ytp[:], bg_sb[:])

                    for ct in range(2):
                        op = o_ps.tile([128, NT], F32)
                        nc.tensor.matmul(
                            op[:],
                            wout_sb[:, ct * 128:(ct + 1) * 128],
                            yt_sb[:],
                            start=True, stop=True,
                        )
                        obn = ostage.tile([128, NT], F32)
                        nc.vector.tensor_scalar(
                            obn[:], op[:],
                            bnscale_sb[:, ct:ct + 1], bnshift_sb[:, ct:ct + 1],
                            op0=MULT, op1=ADD,
                        )
                        ores = ostage.tile([128, NT], F32, name="ores")
                        nc.vector.tensor_tensor(
                            ores[:], obn[:], x_r[ct][:, ntsl].bitcast(F32), op=ADD
                        )
                        nc.sync.dma_start(out[ct * 128:(ct + 1) * 128, ntsl], ores[:])

    nc.finalize()
    return nc


_NC_CACHE = None


def _get_nc():
    global _NC_CACHE
    if _NC_CACHE is None:
        _NC_CACHE = build_nc()
    return _NC_CACHE


def _prepare_in_maps(inputs):
    x = np.ascontiguousarray(np.asarray(inputs["x"], dtype=np.float32)).reshape(B, C, N)
    wth = np.ascontiguousarray(np.asarray(inputs["w_theta"], np.float32).T)
    wph = np.ascontiguousarray(np.asarray(inputs["w_phi"], np.float32).T)
    wg = np.ascontiguousarray(np.asarray(inputs["w_g"], np.float32).T).astype(ml_dtypes.bfloat16)
    wout = np.ascontiguousarray(np.asarray(inputs["w_out"], np.float32).T).astype(ml_dtypes.bfloat16)
    bth = np.asarray(inputs["b_theta"], np.float32).reshape(O, 1)
    bph = np.asarray(inputs["b_phi"], np.float32).reshape(O, 1)
    bg = np.asarray(inputs["b_g"], np.float32).reshape(O, 1)
    inv = np.asarray(inputs["bn_gamma"], np.float32) / np.sqrt(
        np.asarray(inputs["bn_var"], np.float32) + BN_EPS)
    shift = (np.asarray(inputs["b_out"], np.float32) * inv
             + np.asarray(inputs["bn_beta"], np.float32)
             - np.asarray(inputs["bn_mean"], np.float32) * inv)
    bnscale = np.ascontiguousarray(inv.reshape(2, 128).T)
    bnshift = np.ascontiguousarray(shift.reshape(2, 128).T)

    shared = dict(wth=wth, wph=wph, wg=wg, wout=wout, bth=bth, bph=bph,
                  bg=bg, bnscale=bnscale, bnshift=bnshift)
    return [dict(shared, xin=np.ascontiguousarray(x[b])) for b in range(B)]


def _install_ntff_shim():
    """This image's antenv lacks axon_hooks; provide it from trn_boot's
    ctypes implementation so trace=True can capture NTFF profiles."""
    import types
    try:
        import antenv.axon_hooks  # noqa: F401
        return
    except ImportError:
        pass
    if "/root/.axon_site" not in sys.path:
        sys.path.insert(0, "/root/.axon_site")
    from trn_agent_boot.trn_boot import _ntff_profile_via_ctypes
    hook = _ntff_profile_via_ctypes("/opt/axon/libaxon_pjrt.so")
    m = types.ModuleType("antenv.axon_hooks")
    m.get_axon_ntff_profile_hook = lambda: hook
    m.set_axon_ntff_profile_hook = lambda h: None
    sys.modules["antenv.axon_hooks"] = m


def run(inputs, trace=False):
    if trace:
        _install_ntff_shim()
    nc = _get_nc()
    in_maps = _prepare_in_maps(inputs)
    res = run_bass_kernel_spmd(nc, in_maps, list(range(B)), trace=trace)
    outs = np.stack([res.results[b]["out"] for b in range(B)])
    return outs.reshape(B, C, 64, 64), res


def kernel(**inputs) -> np.ndarray:
    out, _ = run(inputs)
    return out


if __name__ == "__main__":
    # quick CoreSim check of one core with random-ish data
    from concourse import bass_interp
    rng = np.random.default_rng(0)
    fake = {
        "x": rng.standard_normal((B, C, 64, 64)).astype(np.float32),
        "w_theta": (rng.standard_normal((O, C)) * 0.05).astype(np.float32),
        "b_theta": (rng.standard_normal(O) * 0.05).astype(np.float32),
        "w_phi": (rng.standard_normal((O, C)) * 0.05).astype(np.float32),
        "b_phi": (rng.standard_normal(O) * 0.05).astype(np.float32),
        "w_g": (rng.standard_normal((O, C)) * 0.05).astype(np.float32),
        "b_g": (rng.standard_normal(O) * 0.05).astype(np.float32),
        "w_out": (rng.standard_normal((C, O)) * 0.05).astype(np.float32),
        "b_out": (rng.standard_normal(C) * 0.05).astype(np.float32),
        "bn_gamma": rng.standard_normal(C).astype(np.float32),
        "bn_beta": rng.standard_normal(C).astype(np.float32),
        "bn_mean": rng.standard_normal(C).astype(np.float32),
        "bn_var": rng.uniform(0.5, 1.5, C).astype(np.float32),
    }
    nc = _get_nc()
    in_maps = _prepare_in_maps(fake)
    sim = bass_interp.CoreSim(nc)
    for k, v in in_maps[0].items():
        sim.tensor(k)[:] = v
    sim.simulate()
    got = np.asarray(sim.tensor("out"))

    # numpy reference for core 0
    x0 = fake["x"][0].reshape(C, N)
    th = fake["w_theta"] @ x0 + fake["b_theta"][:, None]
    ph = fake["w_phi"] @ x0 + fake["b_phi"][:, None]
    gg = fake["w_g"] @ x0 + fake["b_g"][:, None]
    s = th.T @ ph
    p = np.exp(s - s.max(1, keepdims=True))
    a = p / p.sum(1, keepdims=True)
    yy = a @ gg.T
    wy = fake["w_out"] @ yy.T + fake["b_out"][:, None]
    inv = fake["bn_gamma"] / np.sqrt(fake["bn_var"] + BN_EPS)
    bn = wy * inv[:, None] + (fake["bn_beta"] - fake["bn_mean"] * inv)[:, None]
    want = x0 + bn
    err = np.abs(got - want).max()
    print("CoreSim absmax err:", err, "rel:", err / np.abs(want).max())
